# revision 27
# baseline (speedup 1.0000x reference)
"""Trainium2 Bass kernel for DynamicGate MoE routing.

Computes, for x [N=65536, H=1024], sim_matrix [H, E=64], gates [E]:
  logits = l2norm(x, rows) @ l2norm(sim_matrix, cols)      (cosine sims)
  thr = sigmoid(gates); pre = logits - thr; gated = relu(pre)
  hard = (pre > 0); rows with no active expert fall back to top-32 of logits
  mask = hard, or top-32 indicator for inactive rows
  probs = softmax over active experts (uniform 1/32 on fallback rows)
Returns (probs, pre, mask), each [N, E] fp32.

Strategy: data-parallel over tokens across 8 NeuronCores (8192 tokens each).
Host pre-normalizes x and sim_matrix (elementwise input prep) and ships x
TRANSPOSED [H, N] so the device streams perfectly contiguous fp32 tiles with
hidden on the partition dim (no on-device transposes; fp32 XBAR-transpose is
unsupported on TRN2).  Per core: 8 supertiles of 1024 tokens; PE fp32 matmul
accumulates logits into one PSUM bank [128, 8x64] (token t = 128*s + 8*p + g);
the 32nd-largest logit per row is found exactly with a bitonic sort of the two
32-element halves + Batcher median-merge (min over max(A_i, B_rev_i)); the
rest is elementwise work split across DVE / ACT / GPSIMD.
"""

import os
import sys

import numpy as np

for _p in ("/opt/trn_rl_repo", "/root/.axon_site/_ro/trn_rl_repo"):
    if os.path.isdir(_p) and _p not in sys.path:
        sys.path.insert(0, _p)

N_TOKENS = 65536
HIDDEN = 1024
E = 64
CORES = 8
TPC = N_TOKENS // CORES      # tokens per core
ST = 1024                    # tokens per supertile
KC = HIDDEN // 128           # k-chunks of the contraction dim
EPS = 1e-12
P = 128


def _legalize_waits(nc, mybir):
    """Split semaphore waits that exceed the ISA struct's sync-wait slots.

    Walrus encodes a limited number of sync-wait commands per instruction
    (observed: 1 for fp32 self-loading Matmult/LDW, <=2 elsewhere).  Tile can
    emit more.  Excess waits move onto same-engine NoOp carriers inserted
    just before the instruction — engines execute in order, so waiting
    earlier on the same engine is equivalent.
    """
    for f in nc.m.functions:
        for bb in f.blocks:
            out = []
            for inst in bb.instructions:
                si = inst.sync_info
                waits = list(si.on_wait) if (si and si.on_wait) else []
                skip = False
                limit = 1
                if not skip and len(waits) > limit:
                    keep = waits[-limit:]
                    for j, w in enumerate(waits[:-limit]):
                        out.append(mybir.InstNoOp(
                            name=f"{inst.name}-wsp{j}",
                            engine=inst.engine,
                            ins=[], outs=[],
                            sync_info=mybir.SyncInfo(
                                on_wait=[w], on_update=[]),
                        ))
                    inst.sync_info = mybir.SyncInfo(
                        on_wait=keep,
                        on_update=list(si.on_update) if si.on_update else [])
                out.append(inst)
            bb.instructions[:] = out


def build_nc(tpc=TPC, reps=1, ablate=()):
    from concourse import bass, mybir
    from concourse.tile import TileContext

    f32 = mybir.dt.float32
    Alu = mybir.AluOpType
    Act = mybir.ActivationFunctionType
    nst = tpc // ST

    nc = bass.Bass()
    xt_d = nc.declare_dram_parameter("xt", [HIDDEN, tpc], f32, isOutput=False)
    smn_d = nc.declare_dram_parameter("smn", [HIDDEN, E], f32, isOutput=False)
    gates_d = nc.declare_dram_parameter("gates", [1, E], f32, isOutput=False)
    o_d = nc.declare_dram_parameter("o", [nst, 3, ST, E], f32, isOutput=True)

    with TileContext(nc) as tc:
        with (
            tc.tile_pool(name="const", bufs=1) as cpool,
            tc.tile_pool(name="xin", bufs=2) as xpool,
            tc.tile_pool(name="ps", bufs=8, space="PSUM") as pspool,
            tc.tile_pool(name="work", bufs=2) as wpool,
            tc.tile_pool(name="small", bufs=2) as spool,
            tc.tile_pool(name="stg", bufs=2) as gpool,
        ):
            # --- constants: smn chunks [128, (k e)], thr broadcast [128, E]
            smn_sb = cpool.tile([P, KC * E], f32, tag="smn")
            nc.sync.dma_start(
                out=smn_sb[:, :].rearrange("p (k e) -> p k e", k=KC),
                in_=smn_d[:, :].rearrange("(k p) e -> p k e", p=P),
            )
            g_sb = cpool.tile([1, E], f32, tag="gates")
            nc.sync.dma_start(out=g_sb[:, :], in_=gates_d[:, :])
            thr1 = cpool.tile([1, E], f32, tag="thr1")
            nc.scalar.activation(thr1[:, :], g_sb[:, :], Act.Sigmoid)
            thrb = cpool.tile([P, E], f32, tag="thrb")
            thr_dram = nc.dram_tensor("thr_scratch", [1, E], f32)
            nc.sync.dma_start(out=thr_dram[:, :], in_=thr1[:, :])
            nc.sync.dma_start(
                out=thrb[:, :], in_=thr_dram[0:1, :].partition_broadcast(P))
            thr_bc = thrb[:, :].unsqueeze(1).broadcast_to((P, 8, E))

            # One PSUM bank per supertile (8 banks total) so no matmul ever
            # needs a PSUM-WAR wait: fp32 self-loading matmuls have a single
            # sync-wait slot in their LDWEIGHTS struct.
            ps_tiles = [
                pspool.tile([P, 8 * E], f32, tag="ps", name=f"ps{i}")
                for i in range(nst)
            ]
            # PE warm-up matmul depending only on the smn DMA, so later
            # matmuls never pair the smn wait with their xt wait.  Its [64,1]
            # output lands in ps_tiles[0] and is overwritten by the first
            # real start=True matmul.
            nc.tensor.matmul(
                ps_tiles[0][0:E, 0:1], smn_sb[:, 0:E], smn_sb[:, 0:1],
                start=True, stop=True, skip_group_check=True)

            class _Null:
                def __getattr__(self, k):
                    return lambda *a, **kw: None

            if "post" in ablate:
                V = G = A2 = _Null()
            else:
                V, G, A2 = nc.vector, nc.gpsimd, nc.scalar

            def supertile_body(s):
                xt_sb = xpool.tile([P, KC * ST], f32, tag="xt", name=f"xt{s}")
                if "din" not in ablate:
                    nc.sync.dma_start(
                        out=xt_sb[:, :].rearrange("p (k t) -> p k t", k=KC),
                        in_=xt_d[:, s * ST:(s + 1) * ST].rearrange(
                            "(k p) t -> p k t", p=P
                        ),
                    )
                else:
                    nc.sync.dma_start(
                        out=xt_sb[:, 0:1], in_=xt_d[0:P, s:s + 1])
                xt_v = xt_sb[:, :].rearrange("p (k t) -> p k t", k=KC)
                smn_v = smn_sb[:, :].rearrange("p (k e) -> p k e", k=KC)

                ps = ps_tiles[s]
                if "mm" not in ablate:
                    for g in range(8):
                        for k in range(KC):
                            nc.tensor.matmul(
                                ps[:, g * E:(g + 1) * E],
                                xt_v[:, k, g::8],
                                smn_v[:, k, :],
                                start=(k == 0),
                                stop=(k == KC - 1),
                            )
                else:
                    nc.tensor.matmul(
                        ps[:, 0:E], xt_v[:, 0, 0::8], smn_v[:, 0, :],
                        start=True, stop=True, skip_group_check=True)
                ps_v = ps[:, :].rearrange("p (g e) -> p g e", g=8)

                stg = gpool.tile([P, 3 * 8 * E], f32, tag="stg")
                stg_v = stg[:, :].rearrange("p (b g e) -> p b g e", b=3, g=8)
                pre_v = stg_v[:, 1, :, :]
                mask_v = stg_v[:, 2, :, :]

                # evict logits PSUM -> SBUF on ACT (walrus allows only one
                # PSUM operand per tensor_tensor; also frees PSUM earlier)
                lg = wpool.tile([P, 8 * E], f32, tag="lg")
                lg_v = lg[:, :].rearrange("p (g e) -> p g e", g=8)
                A2.copy(lg_v, ps_v)

                # pre-activation logits = logits - thr  (also an output)
                nc.vector.tensor_tensor(pre_v, ps_v, thr_bc, Alu.subtract)

                # ---- exact 32nd-largest per 64-row via bitonic sort ----
                sA = wpool.tile([P, 8 * E], f32, tag="sA")
                sB = wpool.tile([P, 8 * E], f32, tag="sB")

                def cmpex_rev(dst, src, sz):
                    vs = src.rearrange("p (n s) -> p n s", s=sz)
                    vd = dst.rearrange("p (n s) -> p n s", s=sz)
                    h = sz // 2
                    V.tensor_tensor(
                        vd[:, :, 0:h], vs[:, :, 0:h],
                        vs[:, :, sz - 1:h - 1:-1], Alu.min)
                    V.tensor_tensor(
                        vd[:, :, h:sz], vs[:, :, h:sz],
                        vs[:, :, h - 1::-1], Alu.max)

                def cmpex_dist(dst, src, sz, d):
                    c = sz // (2 * d)
                    vs = src.rearrange("p (n c w d) -> p n c w d", c=c, w=2, d=d)
                    vd = dst.rearrange("p (n c w d) -> p n c w d", c=c, w=2, d=d)
                    V.tensor_tensor(
                        vd[:, :, :, 0, :], vs[:, :, :, 0, :],
                        vs[:, :, :, 1, :], Alu.min)
                    V.tensor_tensor(
                        vd[:, :, :, 1, :], vs[:, :, :, 1, :],
                        vs[:, :, :, 0, :], Alu.max)

                stages = []
                for L in (1, 2, 3, 4, 5):
                    sz = 1 << L
                    stages.append(("rev", sz, 0))
                    d = sz // 4
                    while d >= 1:
                        stages.append(("dist", sz, d))
                        d //= 2

                src_ap = lg[:, :]
                dsts = [sA, sB]
                for i, (kind, sz, d) in enumerate(stages):
                    dst_ap = dsts[i % 2][:, :]
                    if kind == "rev":
                        cmpex_rev(dst_ap, src_ap, sz)
                    else:
                        cmpex_dist(dst_ap, src_ap, sz, d)
                    src_ap = dst_ap
                # 15 stages -> sorted 32-blocks live in sA
                srt = sA[:, :].rearrange("p (g w s) -> p g w s", g=8, w=2)
                med = sB[:, :].rearrange("p (g e) -> p g e", g=8)[:, :, 0:32]
                V.tensor_tensor(
                    med, srt[:, :, 0, :], srt[:, :, 1, ::-1], Alu.max)
                v32 = spool.tile([P, 8], f32, tag="v32")
                V.tensor_reduce(
                    v32[:, :], med, mybir.AxisListType.X, Alu.min)
                v32_bc = v32[:, :].unsqueeze(2).broadcast_to((P, 8, E))

                fb = wpool.tile([P, 8 * E], f32, tag="fb")
                fb_v = fb[:, :].rearrange("p (g e) -> p g e", g=8)
                V.tensor_tensor(fb_v, lg_v, v32_bc, Alu.is_ge)

                # mask = max(hard, fb * inactive): hard for active rows
                # (hard==0 there otherwise), fb for inactive rows
                hb = wpool.tile([P, 8 * E], f32, tag="hb")
                hb_v = hb[:, :].rearrange("p (g e) -> p g e", g=8)
                V.tensor_scalar(
                    hb_v, pre_v, 0.0, None, op0=Alu.is_gt)
                mp = spool.tile([P, 8], f32, tag="mp")
                V.tensor_reduce(
                    mp[:, :], pre_v, mybir.AxisListType.X, Alu.max)
                inact = spool.tile([P, 8], f32, tag="inact")
                V.tensor_scalar(
                    inact[:, :], mp[:, :], 0.0, None, op0=Alu.is_le)
                inact_bc = inact[:, :].unsqueeze(2).broadcast_to((P, 8, E))
                fi = wpool.tile([P, 8 * E], f32, tag="fi")
                fi_v = fi[:, :].rearrange("p (g e) -> p g e", g=8)
                G.tensor_tensor(fi_v, fb_v, inact_bc, Alu.mult)
                V.tensor_tensor(mask_v, hb_v, fi_v, Alu.max)

                # softmax over active experts:
                #   m = rowmax(gated * mask); e = exp(gated - m) * mask
                gated = wpool.tile([P, 8 * E], f32, tag="gated")
                gated_v = gated[:, :].rearrange("p (g e) -> p g e", g=8)
                A2.activation(gated_v, pre_v, Act.Relu)
                gm = wpool.tile([P, 8 * E], f32, tag="gm")
                gm_v = gm[:, :].rearrange("p (g e) -> p g e", g=8)
                G.tensor_tensor(gm_v, gated_v, mask_v, Alu.mult)
                m8 = spool.tile([P, 8], f32, tag="m8")
                V.tensor_reduce(
                    m8[:, :], gm_v, mybir.AxisListType.X, Alu.max)
                m8_bc = m8[:, :].unsqueeze(2).broadcast_to((P, 8, E))
                dx = wpool.tile([P, 8 * E], f32, tag="dx")
                dx_v = dx[:, :].rearrange("p (g e) -> p g e", g=8)
                G.tensor_tensor(dx_v, gated_v, m8_bc, Alu.subtract)
                ex = wpool.tile([P, 8 * E], f32, tag="ex")
                ex_v = ex[:, :].rearrange("p (g e) -> p g e", g=8)
                A2.activation(ex_v, dx_v, Act.Exp)
                em = wpool.tile([P, 8 * E], f32, tag="em")
                em_v = em[:, :].rearrange("p (g e) -> p g e", g=8)
                G.tensor_tensor(em_v, ex_v, mask_v, Alu.mult)
                s8 = spool.tile([P, 8], f32, tag="s8")
                V.tensor_reduce(
                    s8[:, :], em_v, mybir.AxisListType.X, Alu.add)
                r8 = spool.tile([P, 8], f32, tag="r8")
                V.reciprocal(r8[:, :], s8[:, :])
                r8_bc = r8[:, :].unsqueeze(2).broadcast_to((P, 8, E))
                G.tensor_tensor(stg_v[:, 0, :, :], em_v, r8_bc, Alu.mult)

                nc.sync.dma_start(
                    out=o_d[s].rearrange("b (p g) e -> p b g e", p=P),
                    in_=stg_v,
                )

            if reps == 1:
                for s in range(nst):
                    supertile_body(s)
            else:
                # device-side repeat loop for wall-clock benchmarking:
                # the body is idempotent, so re-running it reproduces the
                # same outputs while exposing steady-state throughput.
                with tc.For_i(
                    0, reps, 1,
                    hint_engines=(
                        mybir.EngineType.PE, mybir.EngineType.DVE,
                        mybir.EngineType.Activation, mybir.EngineType.Pool,
                    ),
                ):
                    for s in range(nst):
                        supertile_body(s)
    _legalize_waits(nc, mybir)
    return nc


def _preprocess(x, sim_matrix, gates):
    x = np.asarray(x, dtype=np.float32)
    sm = np.asarray(sim_matrix, dtype=np.float32)
    g = np.asarray(gates, dtype=np.float32)
    xn = x / np.maximum(
        np.sqrt(np.sum(x * x, axis=1, keepdims=True, dtype=np.float32)), EPS)
    smn = sm / np.maximum(
        np.sqrt(np.sum(sm * sm, axis=0, keepdims=True, dtype=np.float32)), EPS)
    xt = np.ascontiguousarray(xn.T.astype(np.float32))
    return xt, np.ascontiguousarray(smn), g.reshape(1, E)


def kernel(x, sim_matrix, gates, trace=False, tmpdir=None):
    from concourse.bass_utils import run_bass_kernel_spmd

    xt, smn, g = _preprocess(x, sim_matrix, gates)
    nc = build_nc(TPC)
    in_maps = []
    for c in range(CORES):
        shard = np.ascontiguousarray(xt[:, c * TPC:(c + 1) * TPC])
        in_maps.append({"xt": shard, "smn": smn, "gates": g})
    res = run_bass_kernel_spmd(
        nc, in_maps, list(range(CORES)), trace=trace, tmpdir=tmpdir)
    kernel._last_results = res

    probs = np.empty((N_TOKENS, E), dtype=np.float32)
    pre = np.empty((N_TOKENS, E), dtype=np.float32)
    mask = np.empty((N_TOKENS, E), dtype=np.float32)
    for c in range(CORES):
        o = res.results[c]["o"]          # [nst, 3, ST, E]
        lo, hi = c * TPC, (c + 1) * TPC
        probs[lo:hi] = o[:, 0].reshape(TPC, E)
        pre[lo:hi] = o[:, 1].reshape(TPC, E)
        mask[lo:hi] = o[:, 2].reshape(TPC, E)
    return probs, pre, mask
